# revision 48
# baseline (speedup 1.0000x reference)
"""Trainium2 Bass kernel for GNN message-passing attention block.

Strategy: shard queries (n axis) 8 ways; all matmuls in bf16. On device:
  GroupNorm -> Q/K/V projections (PE) -> dense per-head scores, with the
  score PSUM pre-initialized to log(M) via an identity matmul so a single
  exp (ACT) yields exp(S)*M directly (M = host-precomputed multiplicity
  mask; head-parity splits the concurrently-running row-tiled matmuls
  across two PSUM banks) -> V^T @ A matmul with an appended ones-column
  giving both the weighted sum and the softmax normalizer Z (PE) ->
  divide -> output projection + residual.

The sparse gather/scatter of the reference collapses into the dense mask M
because duplicate neighbor indices share the same score: their combined
softmax weight is multiplicity * exp(s) / Z.

Hardware constraints honored (found empirically; CoreSim does not model
them): two matmuls whose auto-derived tile_positions occupy disjoint PE
row-groups run CONCURRENTLY, so they must never target the same PSUM bank.
"""

import sys

if "/opt/trn_rl_repo" not in sys.path:
    sys.path.insert(0, "/opt/trn_rl_repo")

import numpy as np

import concourse.bacc as bacc
import concourse.mybir as mybir
import concourse.tile as tile
from contextlib import ExitStack

F32 = mybir.dt.float32
BF16 = mybir.dt.bfloat16
AF = mybir.ActivationFunctionType
ALU = mybir.AluOpType
AX = mybir.AxisListType

B, C, N, K, H, DH, NG = 2, 512, 1024, 64, 8, 64, 32
NQ = 128            # queries per core
NCHUNK = C // 128   # 4 channel chunks
NJC = N // 128      # 8 key-node chunks
EPS = 1e-6
GSIZE = (C // NG) * N  # elements per group = 16*1024

_CACHE = {}


def _emit(tc, nc, t):
    """Emit the per-core program, phase-major across both batches so each
    ACT function table loads once and engines pipeline across batches."""
    ctx = t["ctx"]
    P = 128

    wpool = ctx.enter_context(tc.tile_pool(name="weights", bufs=1))
    xpool = ctx.enter_context(tc.tile_pool(name="x", bufs=1))
    hpool = ctx.enter_context(tc.tile_pool(name="h", bufs=1))
    kvpool = ctx.enter_context(tc.tile_pool(name="kv", bufs=1))
    apool = ctx.enter_context(tc.tile_pool(name="attn", bufs=1))
    spool = ctx.enter_context(tc.tile_pool(name="scratch", bufs=1))
    jpool = ctx.enter_context(tc.tile_pool(name="junk", bufs=2))
    smallp = ctx.enter_context(tc.tile_pool(name="small", bufs=2))
    opool = ctx.enter_context(tc.tile_pool(name="out", bufs=1))
    ypool = ctx.enter_context(tc.tile_pool(name="ybuf", bufs=2))
    pp = ctx.enter_context(tc.tile_pool(name="psum", bufs=2, space="PSUM"))
    scp = ctx.enter_context(tc.tile_pool(name="psumsc", bufs=2, space="PSUM"))
    avp = ctx.enter_context(tc.tile_pool(name="psumav", bufs=2, space="PSUM"))

    # ---- DMA loads: few big transfers on the SP queue, in the order the
    # consumers need them (consts -> x/xq b0 -> wq/wk -> x/xq b1 -> wv/wo
    # -> masks), so compute starts as soon as the first arrives ----
    cg = wpool.tile([P, 28], F32)  # bq|bk|gamma|beta|bo_eff|gfwd
    gbwd = wpool.tile([8, P], F32)
    nc.sync.dma_start(cg[:], t["constsg"][:, :])
    nc.sync.dma_start(gbwd[:], t["gbwd"][:, :])
    consts = cg[:, 0:20]
    gfwd = cg[:, 20:28]

    xbig = [xpool.tile([P, NCHUNK * N], F32, tag=f"x{b}", name=f"x{b}")
            for b in range(B)]
    xsb = [[xbig[b][:, m * N:(m + 1) * N] for m in range(NCHUNK)]
           for b in range(B)]
    # host permutes nodes per core so this core's queries are columns 0:NQ
    xq = [[xsb[b][m][:, 0:NQ] for m in range(NCHUNK)] for b in range(B)]
    wbig = {}
    for w in ("wqT", "wkT", "wvT", "woT"):
        wbig[w] = wpool.tile([P, NCHUNK * 512], BF16, tag=w, name=w)
    wqT = [wbig["wqT"][:, i * 512:(i + 1) * 512] for i in range(NCHUNK)]
    wkT = [wbig["wkT"][:, i * 512:(i + 1) * 512] for i in range(NCHUNK)]
    wvT = [wbig["wvT"][:, i * 512:(i + 1) * 512] for i in range(NCHUNK)]
    woT = [wbig["woT"][:, i * 512:(i + 1) * 512] for i in range(NCHUNK)]
    maskbig = wpool.tile([P, NJC * 512], BF16, tag="mask", name="maskbig")
    msb = [maskbig[:, jc * 512:(jc + 1) * 512] for jc in range(NJC)]

    def r3(ap, inner):
        return ap.rearrange("(m p) n -> p m n", p=P)

    xr = [t["x"][b].rearrange("(m p) n -> p m n", p=P) for b in range(B)]
    # batch-0 x in four chunks so GN stats start after the first ~1.5us
    for m in range(NCHUNK):
        nc.sync.dma_start(xsb[0][m], xr[0][:, m, :])
    nc.sync.dma_start(wbig["wqT"][:].rearrange("p (m c) -> p m c", m=NCHUNK),
                      r3(t["wqT"][:, :], 512))
    nc.sync.dma_start(wbig["wkT"][:].rearrange("p (m c) -> p m c", m=NCHUNK),
                      r3(t["wkT"][:, :], 512))
    nc.sync.dma_start(xbig[1][:].rearrange("p (m n) -> p m n", m=NCHUNK),
                      xr[1])
    nc.sync.dma_start(wbig["wvT"][:].rearrange("p (m c) -> p m c", m=NCHUNK),
                      r3(t["wvT"][:, :], 512))
    nc.sync.dma_start(wbig["woT"][:].rearrange("p (m c) -> p m c", m=NCHUNK),
                      r3(t["woT"][:, :], 512))
    nc.sync.dma_start(maskbig[:].rearrange("p (j c) -> p j c", j=NJC),
                      t["mmask"][:, :, :].rearrange("j p c -> p j c"))
    from concourse import masks
    ident = wpool.tile([P, P], BF16)
    masks.make_identity(nc, ident[:])
    zrow = wpool.tile([DH + 1, DH], BF16)   # row 64 ones, rest zero: picks Z
    nc.gpsimd.memset(zrow[:], 0.0)
    nc.gpsimd.memset(zrow[DH:DH + 1, :], 1.0)

    # ---- phase helpers; emission order below is hand-scheduled so no
    # engine FIFO stalls on the other batch's data ----
    ga, gb = [None] * B, [None] * B
    hsb = [[hpool.tile([P, N], BF16, tag=f"h{b}{m}", name=f"h{b}{m}")
            for m in range(NCHUNK)] for b in range(B)]
    hq = [[hsb[b][m][:, 0:NQ] for m in range(NCHUNK)] for b in range(B)]
    qsb = [[kvpool.tile([P, NQ], BF16, tag=f"q{b}{mo}", name=f"q{b}{mo}")
            for mo in range(NCHUNK)] for b in range(B)]
    ksb = [[kvpool.tile([P, N], BF16, tag=f"k{b}{mo}", name=f"k{b}{mo}")
            for mo in range(NCHUNK)] for b in range(B)]
    vT = [[kvpool.tile([P, H * (DH + 1)], BF16, tag=f"vT{b}{jc}", name=f"vT{b}{jc}")
           for jc in range(NJC)] for b in range(B)]
    asb = [[apool.tile([P, H * NQ], BF16, tag=f"a{b}{jc}", name=f"a{b}{jc}")
            for jc in range(NJC)] for b in range(B)]

    def gn(b):
        """stats (ACT squares + DVE sums), group matmuls, per-channel scale/
        shift, and the normalize-apply split across DVE/GpSimd."""
        s = smallp.tile([P, 8], F32, tag=f"ssq{b}", name=f"ssq{b}")
        for m in range(NCHUNK):
            sqjunk = jpool.tile([P, N], BF16, tag="sqjunk", name="sqjunk")
            nc.scalar.activation(sqjunk[:], xsb[b][m][:], AF.Square,
                                 accum_out=s[:, 4 + m:5 + m])
            nc.vector.tensor_reduce(s[:, m:m + 1], xsb[b][m][:], AX.X, ALU.add)
        gnp = pp.tile([P, 512], F32, tag="mm", name="gnp")
        nc.tensor.matmul(gnp[0:8, 0:8], gfwd[:], s[:], start=True, stop=True)
        mu = smallp.tile([8, 8], F32, tag="mu", name="mu")
        nc.vector.tensor_scalar_mul(mu[:], gnp[0:8, 0:8], 1.0 / GSIZE)
        var = smallp.tile([8, 4], F32, tag="var", name="var")
        nc.vector.tensor_tensor(var[:], mu[:, 0:4], mu[:, 0:4], ALU.mult)
        nc.vector.tensor_tensor(var[:], mu[:, 4:8], var[:], ALU.subtract)
        sd = smallp.tile([8, 4], F32, tag="sd", name="sd")
        nc.vector.tensor_scalar_add(sd[:], var[:], EPS)
        sdq = smallp.tile([8, 4], F32, tag="sdq", name="sdq")
        nc.scalar.activation(sdq[:], sd[:], AF.Sqrt)
        rs = smallp.tile([8, 4], F32, tag="rs", name="rs")
        nc.vector.reciprocal(rs[:], sdq[:])
        bcp = pp.tile([P, 512], F32, tag="mm", name="bcp")
        nc.tensor.matmul(bcp[:, 0:4], gbwd[:], mu[:, 0:4], start=True, stop=True)
        nc.tensor.matmul(bcp[:, 4:8], gbwd[:], rs[:], start=True, stop=True)
        a_ = smallp.tile([P, 4], F32, tag=f"ga{b}", name=f"ga{b}")
        b_ = smallp.tile([P, 4], F32, tag=f"gb{b}", name=f"gb{b}")
        nc.vector.tensor_tensor(a_[:], consts[:, 8:12], bcp[:, 4:8], ALU.mult)
        nc.vector.tensor_tensor(b_[:], bcp[:, 0:4], a_[:], ALU.mult)
        nc.vector.tensor_tensor(b_[:], consts[:, 12:16], b_[:], ALU.subtract)
        ga[b], gb[b] = a_, b_
        for m in range(NCHUNK):
            if m == 2:
                nc.scalar.activation(hsb[b][m][:], xsb[b][m][:], AF.Identity,
                                     scale=a_[:, m:m + 1], bias=b_[:, m:m + 1])
            else:
                eng = nc.gpsimd if m == 1 else nc.vector
                eng.tensor_scalar(hsb[b][m][:], xsb[b][m][:],
                                  a_[:, m:m + 1], b_[:, m:m + 1],
                                  ALU.mult, ALU.add)

    def qproj(b):
        pq = pp.tile([P, 512], F32, tag="mm", name="pq")
        for mo in range(NCHUNK):
            osl = slice(mo * 128, (mo + 1) * 128)
            for ki in range(NCHUNK):
                nc.tensor.matmul(pq[:, mo * NQ:(mo + 1) * NQ],
                                 wqT[ki][:, osl], hq[b][ki],
                                 start=(ki == 0), stop=(ki == NCHUNK - 1))
        for mo in range(NCHUNK):
            nc.vector.tensor_scalar_add(qsb[b][mo][:],
                                        pq[:, mo * NQ:(mo + 1) * NQ],
                                        consts[:, mo:mo + 1])

    def kproj_steps(b):
        for mo in range(NCHUNK):
            osl = slice(mo * 128, (mo + 1) * 128)
            for nt in range(2):
                nsl = slice(nt * 512, (nt + 1) * 512)
                pk = pp.tile([P, 512], F32, tag="mm", name="pk")
                for ki in range(NCHUNK):
                    nc.tensor.matmul(pk[:], wkT[ki][:, osl], hsb[b][ki][:, nsl],
                                     start=(ki == 0), stop=(ki == NCHUNK - 1))
                nc.vector.tensor_scalar_add(ksb[b][mo][:, nsl], pk[:],
                                            consts[:, 4 + mo:5 + mo])
                yield

    def vproj_steps(b):
        # V^T per node-chunk: [j, h*65 + d], ones at d=64 (softmax normalizer)
        for jc in range(NJC):
            jsl = slice(jc * 128, (jc + 1) * 128)
            pv = pp.tile([P, 512], F32, tag="mm", name="pv")
            for ki in range(NCHUNK):
                nc.tensor.matmul(pv[:], hsb[b][ki][:, jsl], wvT[ki][:],
                                 start=(ki == 0), stop=(ki == NCHUNK - 1))
            vt3 = vT[b][jc][:].rearrange("p (h c) -> p h c", h=H)
            if jc % 2:
                nc.scalar.activation(vt3[:, :, 0:DH],
                                     pv[:].rearrange("p (h d) -> p h d", h=H),
                                     AF.Identity)
            else:
                nc.vector.tensor_copy(vt3[:, :, 0:DH],
                                      pv[:].rearrange("p (h d) -> p h d", h=H))
            nc.gpsimd.memset(vt3[:, :, DH:DH + 1], 1.0)
            yield

    def scores_steps(b):
        """PSUM initialized with log(M) via identity matmul, per-head K^T Q
        accumulates on top, one exp gives exp(S)*M directly.  asb column
        layout is parity-major: head h = 2m+p at cols (h%2)*512+(h//2)*128.
        Even heads live at partitions 0:64 of ksb/qsb (PE row-groups 0-1),
        odd heads at 64:128 (row-groups 2-3); the two concurrently-running
        streams write separate PSUM banks."""
        for jc in range(NJC):
            jsl = slice(jc * 128, (jc + 1) * 128)
            psE = scp.tile([P, 512], F32, tag="scE", name="psE")
            psO = scp.tile([P, 512], F32, tag="scO", name="psO")
            nc.tensor.matmul(psE[:], ident[:], msb[jc], start=True, stop=False)
            nc.tensor.matmul(psO[:], ident[:], msb[jc], start=True, stop=False)
            for m in range(NCHUNK):
                qsl = slice(m * NQ, (m + 1) * NQ)
                nc.tensor.matmul(psE[:, qsl], ksb[b][m][0:64, jsl],
                                 qsb[b][m][0:64, :],
                                 start=False, stop=(m == NCHUNK - 1))
                nc.tensor.matmul(psO[:, qsl], ksb[b][m][64:128, jsl],
                                 qsb[b][m][64:128, :],
                                 start=False, stop=(m == NCHUNK - 1))
            nc.scalar.activation(asb[b][jc][:, 0:512], psE[:], AF.Exp)
            nc.scalar.activation(asb[b][jc][:, 512:1024], psO[:], AF.Exp)
            yield

    def av_steps(b):
        """Per head: AV matmul accumulating over node chunks (appended ones
        row gives Z); four heads share one PSUM bank in quarters (AV matmuls
        are full-contraction, hence serial on PE, and the bank is only read
        after all 32 MMs).  After each half: broadcast Z over the dh
        partitions via the zrow matmul, invert with 64 DVE lanes, GpSimd
        assembles o, and the output projection for the finished osb chunks
        starts immediately (ki-outer accumulation)."""
        opre = spool.tile([DH + 1, H * NQ], BF16, tag=f"opre{b}", name=f"opre{b}")
        zinv = spool.tile([DH, H * NQ], BF16, tag=f"zinv{b}", name=f"zinv{b}")
        osb = [opool.tile([P, NQ], BF16, tag=f"o{b}{mo}", name=f"o{b}{mo}")
               for mo in range(NCHUNK)]
        ybig = ypool.tile([P, NCHUNK * NQ], F32, tag="y", name="ybig")
        py = pp.tile([P, 512], F32, tag="mm", name="py")
        for nt in range(2):
            po4 = avp.tile([DH + 1, 512], F32, tag="po", name="po4")
            nsl = slice(nt * 512, (nt + 1) * 512)
            for hh in range(4):
                h = nt * 4 + hh
                p_, m_ = h % 2, h // 2
                acol = slice(p_ * 512 + m_ * 128, p_ * 512 + m_ * 128 + 128)
                for jc in range(NJC):
                    nc.tensor.matmul(po4[:, hh * NQ:(hh + 1) * NQ],
                                     vT[b][jc][:, h * 65:(h + 1) * 65],
                                     asb[b][jc][:, acol],
                                     start=(jc == 0), stop=(jc == NJC - 1))
                yield
            nc.vector.tensor_copy(opre[:, nsl], po4[:])
            zb = pp.tile([P, 512], F32, tag="mm", name="zb")
            nc.tensor.matmul(zb[0:DH, :], zrow[:], opre[:, nsl],
                             start=True, stop=True)
            with nc.allow_low_precision(reason="1/Z in bf16 is fine"):
                nc.vector.reciprocal(zinv[:, nsl], zb[0:DH, :])
            for hh in range(4):
                h = nt * 4 + hh
                mo, poff = h // 2, (h % 2) * 64
                hc = h * NQ
                nc.gpsimd.tensor_tensor(
                    osb[mo][poff:poff + 64, :],
                    opre[0:DH, hc:hc + NQ],
                    zinv[0:DH, hc:hc + NQ],
                    ALU.mult)
            yield
        for mo in range(NCHUNK):
            osl = slice(mo * 128, (mo + 1) * 128)
            for ki in range(NCHUNK):
                nc.tensor.matmul(py[:, mo * NQ:(mo + 1) * NQ],
                                 woT[ki][:, osl], osb[ki][:],
                                 start=(ki == 0), stop=(ki == NCHUNK - 1))
        for mo in range(NCHUNK):
            nc.vector.scalar_tensor_tensor(ybig[:, mo * NQ:(mo + 1) * NQ],
                                           py[:, mo * NQ:(mo + 1) * NQ],
                                           consts[:, 16 + mo:17 + mo],
                                           xq[b][mo], ALU.add, ALU.add)
        nc.scalar.dma_start(t["y"][b].rearrange("(m p) q -> p m q", p=P),
                            ybig[:].rearrange("p (m q) -> p m q", m=NCHUNK))
        yield

    def drain(*gens):
        """Round-robin the generators so their PE work interleaves: a step
        whose inputs are still in flight no longer blocks independent work
        queued behind it in the engine FIFOs."""
        gens = list(gens)
        while gens:
            for g in list(gens):
                if next(g, "done") == "done":
                    gens.remove(g)

    # ---- hand-scheduled emission order ----
    gn(0)
    qproj(0)
    drain(kproj_steps(0))
    gn(1)
    drain(vproj_steps(0))
    drain(scores_steps(0))
    qproj(1)
    drain(kproj_steps(1))
    drain(vproj_steps(1))
    drain(scores_steps(1))
    drain(av_steps(0))
    drain(av_steps(1))


def _build():
    nc = bacc.Bacc("TRN2", target_bir_lowering=False, debug=False, num_devices=8)
    t = {}
    t["x"] = nc.dram_tensor("x", [B, C, N], F32, kind="ExternalInput").ap()
    t["mmask"] = nc.dram_tensor("mmask", [NJC, 128, 512], BF16,
                                kind="ExternalInput").ap()
    for w in ("wqT", "wkT", "wvT", "woT"):
        t[w] = nc.dram_tensor(w, [C, C], BF16, kind="ExternalInput").ap()
    t["constsg"] = nc.dram_tensor("constsg", [128, 28], F32, kind="ExternalInput").ap()
    t["gbwd"] = nc.dram_tensor("gbwd", [8, 128], F32, kind="ExternalInput").ap()
    t["y"] = nc.dram_tensor("y", [B, C, NQ], F32, kind="ExternalOutput").ap()
    with tile.TileContext(nc) as tc, ExitStack() as ctx:
        t["ctx"] = ctx
        _emit(tc, nc, t)
    nc.compile()
    return nc


def _prep_inputs(inputs):
    import ml_dtypes
    bf16 = ml_dtypes.bfloat16

    x = np.ascontiguousarray(np.asarray(inputs["x"], dtype=np.float32))
    idx = np.asarray(inputs["attend_idx"]).astype(np.int64)
    vm = np.asarray(inputs["valid_mask"]).astype(np.float32)
    wq = np.asarray(inputs["wq"], dtype=np.float32)
    wk = np.asarray(inputs["wk"], dtype=np.float32)
    wv = np.asarray(inputs["wv"], dtype=np.float32)
    wo = np.asarray(inputs["wo"], dtype=np.float32)
    bq = np.asarray(inputs["bq"], dtype=np.float32)
    bk = np.asarray(inputs["bk"], dtype=np.float32)
    bv = np.asarray(inputs["bv"], dtype=np.float32)
    bo = np.asarray(inputs["bo"], dtype=np.float32)
    gamma = np.asarray(inputs["gn_gamma"], dtype=np.float32)
    beta = np.asarray(inputs["gn_beta"], dtype=np.float32)

    cols = np.arange(C)
    perm = (cols % DH) * H + cols // DH   # wo_perm[:, h*64+d] = wo[:, d*8+h]
    wo_perm = wo[:, perm]
    bo_eff = bo + wo_perm @ bv

    def colmajor(v):
        return np.ascontiguousarray(v.reshape(NCHUNK, 128).T)

    consts = np.concatenate(
        [colmajor(v) for v in (bq, bk, gamma, beta, bo_eff)], axis=1)
    gfwd = np.zeros((128, 8), np.float32)
    gfwd[np.arange(128), np.arange(128) // 16] = 1.0
    gbwd = np.ascontiguousarray(gfwd.T)

    shared = {
        "x": x,
        "wqT": np.ascontiguousarray(wq.T.astype(bf16)),
        "wkT": np.ascontiguousarray(wk.T.astype(bf16)),
        "wvT": np.ascontiguousarray(wv.T.astype(bf16)),
        "woT": np.ascontiguousarray(wo_perm.T.astype(bf16)),
        "constsg": np.ascontiguousarray(
            np.concatenate([consts, gfwd], axis=1)),
        "gbwd": gbwd,
    }
    in_maps = []
    for r in range(8):
        qs = slice(r * NQ, (r + 1) * NQ)
        Mr = np.zeros((N, NQ), np.float32)
        np.add.at(Mr, (idx[qs].ravel(), np.repeat(np.arange(NQ), K)),
                  vm[qs].ravel())
        # permute nodes so this core's queries are columns 0:NQ; the mask
        # rows follow the same permutation (three contiguous slices)
        m = dict(shared)
        m["x"] = np.concatenate(
            [x[:, :, qs], x[:, :, :qs.start], x[:, :, qs.stop:]], axis=2)
        logM = np.where(Mr > 0, np.log(np.maximum(Mr, 1e-30)), -60000.0)
        logM = np.concatenate([logM[qs], logM[:qs.start], logM[qs.stop:]],
                              axis=0)
        logM4 = np.tile(logM.reshape(NJC, 128, 1, NQ), (1, 1, 4, 1))
        m["mmask"] = np.ascontiguousarray(
            logM4.reshape(NJC, 128, 512).astype(bf16))
        in_maps.append(m)
    return in_maps


def _get_runner(n_cores=8):
    """Build (once) a cached jitted SPMD executor mirroring
    bass2jax.run_bass_via_pjrt, so repeated calls don't re-trace."""
    if "runner" in _CACHE:
        return _CACHE["runner"]
    if "nc" not in _CACHE:
        _CACHE["nc"] = _build()
    nc = _CACHE["nc"]
    import jax
    from jax.sharding import Mesh, PartitionSpec
    from jax.experimental.shard_map import shard_map
    from concourse import bass2jax
    import concourse.mybir as _mybir

    bass2jax.install_neuronx_cc_hook()
    part_name = nc.partition_id_tensor.name if nc.partition_id_tensor else None
    in_names, out_names, out_avals, zero_outs = [], [], [], []
    for alloc in nc.m.functions[0].allocations:
        if not isinstance(alloc, _mybir.MemoryLocationSet):
            continue
        name = alloc.memorylocations[0].name
        if alloc.kind == "ExternalInput":
            if name != part_name:
                in_names.append(name)
        elif alloc.kind == "ExternalOutput":
            shape = tuple(alloc.tensor_shape)
            dtype = _mybir.dt.np(alloc.dtype)
            out_names.append(name)
            out_avals.append(jax.core.ShapedArray(shape, dtype))
            zero_outs.append(np.zeros(shape, dtype))
    n_params = len(in_names)
    n_outs = len(out_avals)
    all_names = in_names + out_names
    if part_name is not None:
        all_names = all_names + [part_name]
    donate = tuple(range(n_params, n_params + n_outs))

    def _body(*args):
        operands = list(args)
        if part_name is not None:
            operands.append(bass2jax.partition_id_tensor())
        outs = bass2jax._bass_exec_p.bind(
            *operands,
            out_avals=tuple(out_avals),
            in_names=tuple(all_names),
            out_names=tuple(out_names),
            lowering_input_output_aliases=(),
            sim_require_finite=True,
            sim_require_nnan=True,
            nc=nc,
        )
        return tuple(outs)

    devices = jax.devices()[:n_cores]
    mesh = Mesh(np.asarray(devices), ("core",))
    fn = jax.jit(
        shard_map(_body, mesh=mesh,
                  in_specs=(PartitionSpec("core"),) * (n_params + n_outs),
                  out_specs=(PartitionSpec("core"),) * n_outs,
                  check_rep=False),
        donate_argnums=donate, keep_unused=True)

    def run(in_maps, device_inputs=None):
        if device_inputs is None:
            device_inputs = put_inputs(in_maps)
        zo = [np.concatenate([np.zeros_like(z)] * n_cores, axis=0)
              for z in zero_outs]
        outs = fn(*device_inputs, *zo)
        outs = [np.asarray(o) for o in outs]
        split = [np.split(o, n_cores, axis=0) for o in outs]
        return [{name: split[i][c] for i, name in enumerate(out_names)}
                for c in range(n_cores)]

    def put_inputs(in_maps):
        cat = [np.concatenate([np.asarray(in_maps[c][nm])
                               for c in range(n_cores)], axis=0)
               for nm in in_names]
        return [jax.device_put(a) for a in cat]

    _CACHE["runner"] = (run, put_inputs, fn, n_params, n_outs)
    return _CACHE["runner"]


def _sim_fallback(nc, in_maps):
    """Correctness fallback if the PJRT/hardware path errors: run each
    core's shard through CoreSim."""
    from concourse.bass_interp import CoreSim
    results = []
    for m in in_maps:
        sim = CoreSim(nc, require_finite=False)
        for k, v in m.items():
            sim.tensor(k)[:] = v
        sim.simulate(check_with_hw=False)
        results.append({"y": np.array(sim.tensor("y"))})
    return results


def kernel(**inputs):
    in_maps = _prep_inputs(inputs)
    try:
        run, put_inputs, _, _, _ = _get_runner()
        results = run(in_maps)
    except Exception as e:
        sys.stderr.write(f"kernel: hardware path failed ({e!r}); "
                         "falling back to CoreSim\n")
        results = _sim_fallback(_CACHE["nc"], in_maps)
    out = np.concatenate([np.asarray(results[r]["y"]) for r in range(8)],
                         axis=2)
    return np.ascontiguousarray(out.astype(np.float32))
